# revision 44
# baseline (speedup 1.0000x reference)
"""Trainium2 Bass kernel for nn_JastrowFactorGraph (MGCN-style Jastrow factor).

Strategy (8 NeuronCores, pure data parallel over the 512-walker batch):
  64 walkers/core as 32 "sets" of 2 walkers; SBUF partitions = 64 feats x 2
  walkers (64 RBF-k x 2 walkers in the filter stage).

  Phase 1 (per set): distances ship from host as R[4s+r, cell] rows
  {d^2, d} per walker; a K=4 float32r matmul forms -d^2+2*c_k*d per RBF
  center, one Exp activation (bias -c_k^2) yields the fp16 RBF, fp16
  matmuls contract k into the filter pre-activation, two Tanh activations
  write the fp16 filter grids (e-e 30x30 dst-major + e-n 30x10 elec-major;
  Pool transposes e-n to atom-major and zeroes the e-e diagonal).

  Layers: DVE computes only fp16 2x-mode element-wise products
  (messages = h (x) filt).  The segmented scatter-add reduce is folded
  into the layer matmul: for each source index, one PSUM-accumulating
  matmul spanning all sets of a group (210-280 moving cols, hides the
  per-matmul LDWEIGHTS).  The small atom-side aggregation reduces on the
  otherwise idle Pool engine.  tanh(z+b) batches per group on Act;
  h += tanh on DVE.  Readout: free-dim reduces + one matmul + Exp.
"""

import contextlib

import numpy as np

import concourse.bass as bass
import concourse.mybir as mybir
from concourse.bass_utils import run_bass_kernel_spmd

N_CORES = 8
NB = 512
NW = NB // N_CORES      # walkers per core = 64
NSETS = NW // 2         # 32 sets (2 walkers per set)
NE = 30                 # electrons
NA = 10                 # atoms
K = 64                  # RBF size
RBF_CUT = 8.0
NLAYERS = 2

# cell addressing (padded so each argmm chunk sits in one psum bank)
C_EE = 900              # ee cells (j dst-major: cell = 30*j + i)
C_EN = 300              # en cells elec-major: 1024 + 10*e + a
CELLS = 1024 + C_EN     # 1324 (hole [900:1024) is zero in R)
ARG_CHUNKS = ((0, 512), (512, 1024), (1024, 1324))

# F grid per set slot: [0:900) ee (j-major) | [900:1200) en atom-major
# (30a+e) | [1200:1500) en elec-major (10e+a)
FW = 1500
NFSLOT = 24
# H / z / T column order per set: [ee j 0:30 | atoms 30:40 | elecs 40:70]
HW_ = 70

GROUPS = [list(range(0, 7)), list(range(7, 14)), list(range(14, 21)),
          list(range(21, 28)), list(range(28, 30)), list(range(30, 32))]
NG = len(GROUPS)
GRP_OF = {}
for _g, _sets in enumerate(GROUPS):
    for _s in _sets:
        GRP_OF[_s] = _g


def _gl_order():
    out = []
    for t in range(NG + 1):
        if t < NG:
            out.append((t, 0))
        if t - 1 >= 0:
            out.append((t - 1, 1))
    return out


GL = _gl_order()                       # 10 (group, layer) blocks in time order
GLIDX = {gl: i for i, gl in enumerate(GL)}

DT = mybir.dt.float32
DT16 = mybir.dt.float16
DTR = mybir.dt.float32r
ADD = mybir.AluOpType.add

NP0 = 7        # P slots for layer 0 (s % 7 -> group-contiguous)
NP1 = 14       # P slots for layer 1 (s % 14 -> group-contiguous)
NRBUF = 4


def _ap(base, dims):
    return bass.AP(
        tensor=base.tensor,
        offset=base.offset,
        ap=[base.ap[0]] + [[int(s), int(c)] for s, c in dims],
    )


def _pslot(s, l):
    return (s % NP0) * FW if l == 0 else (NP0 + s % NP1) * FW


def _aslot(s, l):
    return (s % NP0) * NA if l == 0 else (NP0 + s % NP1) * NA


def _fslot(s):
    return (s % NFSLOT) * FW


_CACHE = {}


def _build_module():
    nc = bass.Bass()
    AF = mybir.ActivationFunctionType

    inp = {}

    def din(name, shape, dt=DT):
        inp[name] = nc.declare_dram_parameter(name, list(shape), dt,
                                              isOutput=False)

    din("R", [10 * NSETS, CELLS], DTR)
    din("C4", [10, 128], DTR)
    din("CNEG2", [128, 1])
    din("WF_ee", [128, 128], DT16)
    din("WF_en", [128, 128], DT16)
    din("BF_ee", [128, 1])
    din("BF_en", [128, 1])
    for l in range(NLAYERS):
        din(f"WL_ee_{l}", [128, 128], DT16)
        din(f"WL_en_{l}", [128, 128], DT16)
        din(f"BL_ee_{l}", [128, 1])
        din(f"BL_en_{l}", [128, 1])
    din("WR_ee", [128, 2])
    din("WR_en", [128, 2])
    din("H0", [128, HW_], DT16)
    din("BRS", [128, 1])
    y = nc.declare_dram_parameter("y", [2, NSETS], DT, isOutput=True)

    # ---- static schedule ---------------------------------------------------
    # layer-1 muls of group g are interleaved into the set positions of
    # group g+1, three positions late so hadd(g,0) has completed
    l1_after = {}
    for g in range(1, NG):
        prev, cur = GROUPS[g - 1], GROUPS[g]
        for q in range(len(prev)):
            pos = cur[q + 3] if q + 3 < len(cur) else cur[-1]
            l1_after.setdefault(pos, []).append(prev[q])

    # DVE hadd emission positions
    hadd0_at = {GROUPS[g][min(3, len(GROUPS[g]) - 1)]: g - 1
                for g in range(1, NG)}
    hadd1_at = {}
    for g in range(2, NG):
        pos = min(GROUPS[g][0] + 6, NSETS - 1)
        hadd1_at.setdefault(pos, []).append(g - 2)

    # PE z-block triggers (after filtmm of that set)
    pe_blocks = {}
    tail_blocks = []
    for (g, l) in GL:
        t = g + 1 + l
        if t < NG:
            trig = GROUPS[t][0] + 1 + (2 if l == 1 else 0)
            if trig < NSETS:
                pe_blocks.setdefault(trig, []).append((g, l))
            else:
                tail_blocks.append((g, l))
        else:
            tail_blocks.append((g, l))

    # Act group-tanh triggers (two sets after the PE trigger: the z-matmuls
    # drain in chunks across ~2 set positions)
    act_blocks = {}
    act_tail = []
    for s in sorted(pe_blocks):
        for bl in pe_blocks[s]:
            if s + 2 < NSETS:
                act_blocks.setdefault(s + 2, []).append(bl)
            else:
                act_tail.append(bl)
    act_tail.extend(tail_blocks)

    with contextlib.ExitStack() as st:
        ent = st.enter_context
        block = ent(nc.Block())
        sem = lambda n: ent(nc.semaphore(n))
        s_w = sem("s_w")
        s_ra = sem("s_ra")
        s_rb = sem("s_rb")
        s_arg = sem("s_arg")
        s_exp = sem("s_exp")
        s_flt = sem("s_flt")
        s_tee = sem("s_tee")
        s_ten = sem("s_ten")
        s_pl = sem("s_pl")
        s_m0 = sem("s_m0")
        s_m1 = sem("s_m1")
        s_p20 = sem("s_p20")
        s_p21 = sem("s_p21")
        s_zmm = sem("s_zmm")
        s_th2 = sem("s_th2")
        s_had = sem("s_had")
        s_rs = sem("s_rs")
        s_omm = sem("s_omm")
        s_oact = sem("s_oact")
        s_odma = sem("s_odma")

        sb = lambda n, sh, dt=DT: ent(nc.sbuf_tensor(n, sh, dt))
        R_t = [sb(f"R_t{i}", [10, CELLS], DTR) for i in range(NRBUF)]
        C4_t = sb("C4_t", [10, 128], DTR)
        CN_t = sb("CN_t", [128, 1])
        WFe_t = sb("WFe_t", [128, 128], DT16)
        WFn_t = sb("WFn_t", [128, 128], DT16)
        BFe_t = sb("BFe_t", [128, 1])
        BFn_t = sb("BFn_t", [128, 1])
        WL_t = [[sb(f"WLe{l}_t", [128, 128], DT16),
                 sb(f"WLn{l}_t", [128, 128], DT16)] for l in range(NLAYERS)]
        BL_t = [[sb(f"BLe{l}_t", [128, 1]), sb(f"BLn{l}_t", [128, 1])]
                for l in range(NLAYERS)]
        WRe_t = sb("WRe_t", [128, 2])
        WRn_t = sb("WRn_t", [128, 2])
        BRS_t = sb("BRS_t", [128, 1])
        H0_t = sb("H0_t", [128, HW_], DT16)
        RBF_t = [sb("RBF_t0", [128, CELLS], DT16),
                 sb("RBF_t1", [128, CELLS], DT16)]
        F_t = sb("F_t", [128, NFSLOT * FW], DT16)
        P_t = sb("P_t", [128, (NP0 + NP1) * FW], DT16)
        AGT_t = sb("AGT_t", [128, (NP0 + NP1) * NA], DT16)
        H_t = sb("H_t", [128, NSETS * HW_], DT16)
        T_t = sb("T_t", [128, 2 * 7 * HW_], DT16)
        RS_e = sb("RS_e", [128, NSETS])
        RS_n = sb("RS_n", [128, NSETS])
        O_t = sb("O_t", [2, NSETS])

        ps_ab = [ent(nc.psum_tensor("ps_ab0", [128, 1536], DT)),
                 ent(nc.psum_tensor("ps_ab1", [128, 1536], DT))]
        ps_z = ent(nc.psum_tensor("ps_z", [128, 1024], DT))

        n_wdma = 0

        # ---- SP: weight DMAs + even-set R loads ----------------------------
        # staged order: phase-1 weights first so compute starts early; R(0),
        # R(2), R(4) interleaved between weight groups.
        N_W_PE = 4    # C4, CN, WFe, WFn
        N_W_ACT = 6   # + BFe, BFn
        N_W_H0 = 7    # + H0
        N_W_ZB = 15   # + WL/BL both layers

        @block.sync
        def _(sync):
            nonlocal n_wdma

            def wload(dst, name):
                nonlocal n_wdma
                sync.dma_start(out=dst[:], in_=inp[name][:, :]).then_inc(s_w, 16)
                n_wdma += 1

            def rload(s):
                if s >= NRBUF:
                    sync.wait_ge(s_arg, s - NRBUF + 1)
                src = bass.AP(tensor=inp["R"], offset=10 * s * CELLS,
                              ap=[[CELLS, 10], [1, CELLS]])
                sync.dma_start(out=R_t[s % NRBUF][:],
                               in_=src).then_inc(s_ra, 16)

            for s in range(NSETS):
                rload(s)

        # ---- PE ------------------------------------------------------------
        # z-matmuls drain in whole accumulation-group units (interleaving
        # foreign matmuls inside an open PSUM accumulation group corrupts
        # results on hardware)

        @block.tensor
        def _(tensor):
            tensor.wait_ge(s_w, 16 * 2)

            def zblock_ops(g, l):
                """Return drain units: closures emitting whole accumulation
                groups of (g, l)."""
                sets = GROUPS[g]
                ns = len(sets)
                s0 = sets[0]
                bi = GLIDX[(g, l)]
                zo = 512 * (bi % 2)
                po = _pslot(s0, l)
                def first_waits():
                    tensor.wait_ge(s_w, 16 * N_W_ZB)
                    tensor.wait_ge(s_m0 if l == 0 else s_m1, sets[-1] + 1)
                    tensor.wait_ge(s_p20 if l == 0 else s_p21, sets[-1] + 1)
                    if bi >= 2:
                        tensor.wait_ge(s_th2, bi - 1)   # ps_z slot free

                def ee_unit():     # ee source steps -> z[0:30)
                    first_waits()
                    for i in range(NE):
                        mv = _ap(P_t[:, po + i:po + i + 1],
                                 [[FW, ns], [30, NE]])
                        tensor.matmul(
                            _ap(ps_z[:, zo:zo + 1], [[HW_, ns], [1, NE]]),
                            WL_t[l][0][:], mv, start=(i == 0),
                            stop=(i == NE - 1), skip_group_check=True)

                def en_unit():     # en atom-source steps -> z[40:70)
                    for a in range(NA):
                        mv = _ap(P_t[:, po + 1200 + a:po + 1200 + a + 1],
                                 [[FW, ns], [NA, NE]])
                        tensor.matmul(
                            _ap(ps_z[:, zo + 40:zo + 41], [[HW_, ns], [1, NE]]),
                            WL_t[l][1][:], mv, start=(a == 0),
                            stop=(a == NA - 1), skip_group_check=True)
                    ao = _aslot(s0, l)     # atom aggregation -> z[30:40)
                    mv = _ap(AGT_t[:, ao:ao + 1], [[NA, ns], [1, NA]])
                    tensor.matmul(
                        _ap(ps_z[:, zo + 30:zo + 31], [[HW_, ns], [1, NA]]),
                        WL_t[l][1][:], mv, start=True, stop=True,
                        skip_group_check=True).then_inc(s_zmm, 1)

                return [ee_unit, en_unit]

            zq = []

            def drain(n=1):
                for _ in range(min(n, len(zq))):
                    zq.pop(0)()

            def argmm(s):
                tensor.wait_ge(s_ra, 16 * (s + 1))
                if s >= 2:     # region (s%2) free: tanh of s-2 drained it
                    tensor.wait_ge(s_tee, s - 1)
                    tensor.wait_ge(s_ten, s - 1)
                rt = R_t[s % NRBUF]
                for ci, (c0, c1) in enumerate(ARG_CHUNKS):
                    mm = tensor.matmul(
                        ps_ab[s % 2][:, c0:c1], C4_t[:], rt[:, c0:c1],
                        start=True, stop=True, skip_group_check=True)
                    if ci == len(ARG_CHUNKS) - 1:
                        mm.then_inc(s_arg, 1)

            def filtmm(s):
                tensor.wait_ge(s_exp, s + 1)
                if s == 0:
                    tensor.wait_ge(s_w, 16 * N_W_PE)
                for ci, (c0, c1) in enumerate(ARG_CHUNKS):
                    w = WFe_t if c0 < 1024 else WFn_t
                    mm = tensor.matmul(ps_ab[s % 2][:, c0:c1], w[:],
                                       RBF_t[s % 2][:, c0:c1], start=True,
                                       stop=True, skip_group_check=True)
                    if ci == len(ARG_CHUNKS) - 1:
                        mm.then_inc(s_flt, 1)

            argmm(0)
            for s in range(NSETS):
                if s + 1 < NSETS:
                    argmm(s + 1)
                for bl in pe_blocks.get(s, []):
                    zq.extend(zblock_ops(*bl))
                # a z-block's units must stay contiguous and the whole block
                # sandwiched between an argmm and a filtmm (any other
                # adjacency corrupts on HW)
                drain(len(zq))
                filtmm(s)
            for bl in tail_blocks:
                zq.extend(zblock_ops(*bl))
            drain(len(zq))
            tensor.wait_ge(s_rs, NG)
            tensor.wait_ge(s_w, 16 * n_wdma)
            tensor.matmul(ps_z[0:2, 0:NSETS], WRe_t[:], RS_e[:],
                          start=True, stop=False, skip_group_check=True)
            tensor.matmul(ps_z[0:2, 0:NSETS], WRn_t[:], RS_n[:],
                          start=False, stop=True,
                          skip_group_check=True).then_inc(s_omm, 1)

        # ---- Act -----------------------------------------------------------
        @block.scalar
        def _(scalar):
            scalar.wait_ge(s_w, 16 * 2)

            def tanh2(g, l):
                sets = GROUPS[g]
                ns = len(sets)
                bi = GLIDX[(g, l)]
                scalar.wait_ge(s_zmm, bi + 1)
                if bi >= 2:
                    scalar.wait_ge(s_had, bi - 1)   # T slot free
                zo = 512 * (bi % 2)
                to = (bi % 2) * 7 * HW_
                scalar.activation(
                    _ap(T_t[:, to:to + 1], [[HW_, ns], [1, 30]]),
                    _ap(ps_z[:, zo:zo + 1], [[HW_, ns], [1, 30]]),
                    AF.Tanh, bias=BL_t[l][0][:, 0:1], scale=1.0)
                scalar.activation(
                    _ap(T_t[:, to + 30:to + 31], [[HW_, ns], [1, 40]]),
                    _ap(ps_z[:, zo + 30:zo + 31], [[HW_, ns], [1, 40]]),
                    AF.Tanh, bias=BL_t[l][1][:, 0:1],
                    scale=1.0).then_inc(s_th2, 1)

            def expop(s):
                scalar.wait_ge(s_arg, s + 1)
                if s >= 2:
                    scalar.wait_ge(s_flt, s - 1)    # RBF slot consumed
                scalar.activation(RBF_t[s % 2][:], ps_ab[s % 2][:, 0:CELLS],
                                  AF.Exp, bias=CN_t[:, 0:1],
                                  scale=1.0).then_inc(s_exp, 1)

            def tanhs(s):
                scalar.wait_ge(s_flt, s + 1)
                if s == 0:
                    scalar.wait_ge(s_w, 16 * N_W_ACT)
                if s >= NFSLOT:                      # F slot free
                    scalar.wait_ge(s_m1, s - NFSLOT + 1)
                fb = _fslot(s)
                scalar.activation(F_t[:, fb:fb + C_EE],
                                  ps_ab[s % 2][:, 0:C_EE],
                                  AF.Tanh, bias=BFe_t[:, 0:1],
                                  scale=1.0).then_inc(s_tee, 1)
                scalar.activation(F_t[:, fb + 1200:fb + 1500],
                                  ps_ab[s % 2][:, 1024:1324],
                                  AF.Tanh, bias=BFn_t[:, 0:1],
                                  scale=1.0).then_inc(s_ten, 1)

            # tanhs run one position late: during a z-block, Act still has
            # exp(s) plus both tanhs of s-1 (whose filtmm already ran)
            for s in range(NSETS):
                expop(s)
                if s >= 1:
                    tanhs(s - 1)
                for bl in act_blocks.get(s, []):
                    tanh2(*bl)
            tanhs(NSETS - 1)
            for bl in act_tail:
                tanh2(*bl)
            scalar.wait_ge(s_omm, 1)
            scalar.activation(O_t[:], ps_z[0:2, 0:NSETS], AF.Exp,
                              bias=BRS_t[0:2, 0:1],
                              scale=1.0).then_inc(s_oact, 1)

        # ---- DVE -----------------------------------------------------------
        @block.vector
        def _(vector):
            vector.wait_ge(s_w, 16 * N_W_H0)

            def muls(s, l):
                fb = _fslot(s)
                po = _pslot(s, l)
                if l == 0:
                    vector.wait_ge(s_ten, s + 1)
                    vector.wait_ge(s_pl, 2 * (s + 1))
                    if s >= NP0:
                        vector.wait_ge(s_zmm, GLIDX[(GRP_OF[s - NP0], 0)] + 1)
                    H, hb = H0_t, 0
                else:
                    if s >= NP1:
                        vector.wait_ge(s_zmm, GLIDX[(GRP_OF[s - NP1], 1)] + 1)
                    H, hb = H_t, s * HW_
                # ee: P[0:900) = h_ee[i] * F_ee (j-major, i inner)
                vector.tensor_mul(
                    _ap(P_t[:, po:po + 1], [[30, 30], [1, 30]]),
                    _ap(H[:, hb:hb + 1], [[0, 30], [1, 30]]),
                    _ap(F_t[:, fb:fb + 1], [[30, 30], [1, 30]]))
                # en atom-major: P[900:1200) = h_el[e] * F_am (30a+e)
                vector.tensor_mul(
                    _ap(P_t[:, po + 900:po + 901], [[30, NA], [1, 30]]),
                    _ap(H[:, hb + 40:hb + 41], [[0, NA], [1, 30]]),
                    _ap(F_t[:, fb + 900:fb + 901], [[30, NA], [1, 30]]))
                # atom-side aggregation: AGT[a] = sum_e P_am[30a+e]
                ao = _aslot(s, l)
                with nc.allow_low_precision(reason="fp16 message sums"):
                    vector.tensor_reduce(
                        _ap(AGT_t[:, ao:ao + 1], [[1, NA]]),
                        _ap(P_t[:, po + 900:po + 901], [[30, NA], [1, 30]]),
                        mybir.AxisListType.X,
                        ADD).then_inc(s_m0 if l == 0 else s_m1, 1)

            def hadd(g, l):
                sets = GROUPS[g]
                ns = len(sets)
                bi = GLIDX[(g, l)]
                vector.wait_ge(s_th2, bi + 1)
                hb = sets[0] * HW_
                to = (bi % 2) * 7 * HW_
                out = _ap(H_t[:, hb:hb + 1], [[HW_, ns], [1, HW_]])
                tin = _ap(T_t[:, to:to + 1], [[HW_, ns], [1, HW_]])
                if l == 0:
                    h0 = _ap(H0_t[:, 0:1], [[0, ns], [1, HW_]])
                    vector.tensor_add(out, h0, tin).then_inc(s_had, 1)
                else:
                    vector.tensor_add(out, out, tin).then_inc(s_had, 1)
                    vector.tensor_reduce(
                        RS_e[:, sets[0]:sets[0] + ns],
                        _ap(H_t[:, hb:hb + 1], [[HW_, ns], [1, 30]]),
                        mybir.AxisListType.X, ADD)
                    vector.tensor_reduce(
                        RS_n[:, sets[0]:sets[0] + ns],
                        _ap(H_t[:, hb + 30:hb + 31], [[HW_, ns], [1, 40]]),
                        mybir.AxisListType.X, ADD).then_inc(s_rs, 1)

            emitted = set()
            for s in range(NSETS):
                muls(s, 0)
                if s in hadd0_at:
                    hadd(hadd0_at[s], 0)
                    emitted.add((hadd0_at[s], 0))
                for s1 in l1_after.get(s, []):
                    muls(s1, 1)
                for g1 in hadd1_at.get(s, []):
                    hadd(g1, 1)
                    emitted.add((g1, 1))
            # tail
            g_last = NG - 1
            hadd(g_last, 0)
            emitted.add((g_last, 0))
            for s1 in GROUPS[g_last]:
                muls(s1, 1)
            for (g, l) in GL:
                if l == 1 and (g, 1) not in emitted:
                    hadd(g, 1)
                    emitted.add((g, 1))


        # ---- Pool ----------------------------------------------------------
        @block.gpsimd
        def _(gpsimd):
            wl_list = [(C4_t, "C4"), (CN_t, "CNEG2"),
                       (WFe_t, "WF_ee"), (WFn_t, "WF_en"),
                       (BFe_t, "BF_ee"), (BFn_t, "BF_en"), (H0_t, "H0")]
            for l in range(NLAYERS):
                wl_list += [(WL_t[l][0], f"WL_ee_{l}"),
                            (WL_t[l][1], f"WL_en_{l}"),
                            (BL_t[l][0], f"BL_ee_{l}"),
                            (BL_t[l][1], f"BL_en_{l}")]
            wl_list += [(WRe_t, "WR_ee"), (WRn_t, "WR_en"), (BRS_t, "BRS")]
            for dst, name in wl_list:
                gpsimd.dma_start(out=dst[:],
                                 in_=inp[name][:, :]).then_inc(s_w, 16)

            def p2mul(s, l):
                # en elec-major: P[1200:1500) = h_at[a] * F_em (10e+a)
                fb = _fslot(s)
                po = _pslot(s, l)
                if l == 0:
                    H, hb = H0_t, 0
                else:
                    gpsimd.wait_ge(s_had, GLIDX[(GRP_OF[s], 0)] + 1)
                    H, hb = H_t, s * HW_
                if l == 0 and s >= NP0:
                    gpsimd.wait_ge(s_zmm, GLIDX[(GRP_OF[s - NP0], 0)] + 1)
                if l == 1 and s >= NP1:
                    gpsimd.wait_ge(s_zmm, GLIDX[(GRP_OF[s - NP1], 1)] + 1)
                gpsimd.tensor_mul(
                    _ap(P_t[:, po + 1200:po + 1201], [[NA, 30], [1, NA]]),
                    _ap(H[:, hb + 30:hb + 31], [[0, 30], [1, NA]]),
                    _ap(F_t[:, fb + 1200:fb + 1201],
                        [[NA, 30], [1, NA]])).then_inc(s_p20 if l == 0 else s_p21, 1)

            for s in range(1, NSETS, 2):
                for sp in (s - 1, s):
                    if sp >= 11:
                        p2mul(sp - 11, 1)
                    fb = _fslot(sp)
                    gpsimd.wait_ge(s_tee, sp + 1)
                    gpsimd.memset(
                        _ap(F_t[:, fb:fb + 1], [[31, 30]]), 0.0)
                    gpsimd.wait_ge(s_ten, sp + 1)
                    gpsimd.tensor_copy(
                        F_t[:, fb + 900:fb + 1200],
                        _ap(F_t[:, fb + 1200:fb + 1201], [[1, NA], [NA, 30]]))
                    gpsimd.sem_inc(s_pl, 2)
                    p2mul(sp, 0)
            for sp in range(NSETS - 11, NSETS):
                p2mul(sp, 1)
            gpsimd.wait_ge(s_oact, 1)
            gpsimd.dma_start(out=y[:, :], in_=O_t[:]).then_inc(s_odma, 16)
            gpsimd.wait_ge(s_odma, 16)

    return nc


def _host_prep(pos, atoms, emb_ee, wf_ee, bf_ee, wl_ee, bl_ee, wr_ee, br_ee,
               emb_en, wf_en, bf_en, wl_en, bl_en, wr_en, br_en,
               ee_types, en_types):
    f32 = np.float32
    f16 = np.float16
    centers = np.linspace(0.0, RBF_CUT, K).astype(f32)

    xyz = pos.reshape(NB, NE, 3).astype(f32)
    diff = xyz[:, None, :, :] - xyz[:, :, None, :]          # [nb, j, i, 3]
    d_ee = np.sqrt((diff ** 2).sum(-1)).reshape(NB, NE * NE)
    dn = xyz[:, :, None, :] - atoms.astype(f32)[None, None, :, :]
    d_en = np.sqrt((dn ** 2).sum(-1)).reshape(NB, NE * NA)  # [nb, 10e+a]

    def blockdiag(w, dt):
        o = np.zeros((128, 128), dt)
        o[:64, :64] = w
        o[64:, 64:] = w
        return o

    def tf32_round(a):
        b = np.ascontiguousarray(a, np.float32).view(np.uint32)
        b = (b + 0x1000) & np.uint32(0xFFFFE000)
        return b.view(np.float32)

    c2 = (2.0 * centers).astype(f32)
    c2hi = tf32_round(c2)
    c2lo = tf32_round(c2 - c2hi)
    C4 = np.zeros((10, 128), f32)
    for w, sl in ((0, slice(0, 64)), (1, slice(64, 128))):
        C4[5 * w + 0, sl] = -1.0
        C4[5 * w + 1, sl] = -1.0
        C4[5 * w + 2, sl] = c2hi
        C4[5 * w + 3, sl] = c2lo
        C4[5 * w + 4, sl] = c2hi
    CNEG2 = np.tile(-(centers ** 2), 2).reshape(128, 1).astype(f32)

    def rep2(v, dt=f32):
        return np.tile(np.asarray(v, f32).reshape(-1), 2).reshape(128, 1).astype(dt)

    # H0 in [ee 0:30 | atoms 30:40 | elecs 40:70] col order
    h0_ee = emb_ee[ee_types].T.astype(f32)               # [64, 30]
    h0_at = emb_en[en_types[NE:]].T.astype(f32)          # [64, 10]
    h0_el = emb_en[en_types[:NE]].T.astype(f32)          # [64, 30]
    H0_half = np.concatenate([h0_ee, h0_at, h0_el], axis=1)
    H0 = np.concatenate([H0_half, H0_half], axis=0).astype(f16)

    WR_ee = np.zeros((128, 2), f32)
    WR_ee[:64, 0] = wr_ee[:, 0]
    WR_ee[64:, 1] = wr_ee[:, 0]
    WR_en = np.zeros((128, 2), f32)
    WR_en[:64, 0] = wr_en[:, 0]
    WR_en[64:, 1] = wr_en[:, 0]

    const = {
        "C4": C4, "CNEG2": CNEG2,
        "WF_ee": blockdiag(wf_ee, f16), "WF_en": blockdiag(wf_en, f16),
        "BF_ee": rep2(bf_ee), "BF_en": rep2(bf_en),
        "WR_ee": WR_ee, "WR_en": WR_en, "H0": np.ascontiguousarray(H0),
        "BRS": np.full((128, 1), float(br_ee[0]) + float(br_en[0]), f32),
    }
    for l in range(NLAYERS):
        const[f"WL_ee_{l}"] = blockdiag(wl_ee[l], f16)
        const[f"WL_en_{l}"] = blockdiag(wl_en[l], f16)
        const[f"BL_ee_{l}"] = rep2(bl_ee[l])
        const[f"BL_en_{l}"] = rep2(bl_en[l])

    in_maps = []
    for c in range(N_CORES):
        dloc_ee = d_ee[c * NW:(c + 1) * NW]
        dloc_en = d_en[c * NW:(c + 1) * NW]
        R = np.zeros((NSETS, 10, CELLS), f32)
        for w in range(2):
            for sl, dv in ((slice(0, C_EE), dloc_ee[w::2]),
                           (slice(1024, 1324), dloc_en[w::2])):
                d2 = (dv * dv).astype(f32)
                d2hi = tf32_round(d2)
                dhi = tf32_round(dv.astype(f32))
                R[:, 5 * w + 0, sl] = d2hi
                R[:, 5 * w + 1, sl] = tf32_round(d2 - d2hi)
                R[:, 5 * w + 2, sl] = dhi
                R[:, 5 * w + 3, sl] = dhi
                R[:, 5 * w + 4, sl] = tf32_round(dv.astype(f32) - dhi)
        m = dict(const)
        m["R"] = np.ascontiguousarray(R.reshape(10 * NSETS, CELLS))
        in_maps.append(m)
    return in_maps


def kernel(pos, atoms, emb_ee, wf_ee, bf_ee, wl_ee, bl_ee, wr_ee, br_ee,
           emb_en, wf_en, bf_en, wl_en, bl_en, wr_en, br_en,
           ee_src, ee_dst, ee_types, en_src, en_dst, en_types):
    in_maps = _host_prep(
        np.asarray(pos), np.asarray(atoms), np.asarray(emb_ee),
        np.asarray(wf_ee), np.asarray(bf_ee), np.asarray(wl_ee),
        np.asarray(bl_ee), np.asarray(wr_ee), np.asarray(br_ee),
        np.asarray(emb_en), np.asarray(wf_en), np.asarray(bf_en),
        np.asarray(wl_en), np.asarray(bl_en), np.asarray(wr_en),
        np.asarray(br_en), np.asarray(ee_types), np.asarray(en_types))
    if "nc" not in _CACHE:
        _CACHE["nc"] = _build_module()
    res = run_bass_kernel_spmd(_CACHE["nc"], in_maps, list(range(N_CORES)))
    out = np.concatenate(
        [res.results[c]["y"].T.reshape(NW, 1) for c in range(N_CORES)],
        axis=0)
    return out.astype(np.float32)
